# revision 11
# baseline (speedup 1.0000x reference)
"""Concept-whitening layer (Newton-Schulz iterative ZCA + rotation) on 8
Trainium2 NeuronCores.

Strategy (data-parallel over batch N):
  - each core holds 8 of the 64 samples: x_loc [C=256, m_loc=8192] in SBUF
  - per-core uncentered second moment G = x x^T and column-sums s computed
    on TensorE (PE transposes of x feed the G matmuls; a ones-column in the
    transposed tiles makes column 256 of the G psum accumulate s)
  - one AllReduce of [2,128,257] (G|s) across the 8 cores; a prelude
    1-byte AllGather (bir_kernel_barrier) eats the first-collective
    barrier cost concurrently with the local G phase
  - Sigma = G/m - mu mu^T + eps I computed from the reduced stats
    (identical to centered covariance), Newton-Schulz (10 iters) and the
    rotation are replicated on every core; rotation is folded into the
    whitening matrix: out = (R wm)(x - mu) = A x - A mu
  - the whitening+rotation apply and output DMA are local to the shard
Heavy matmuls use float32r (full-rate single-pass PE mode, ~1.6e-4 rel
precision); every tensor feeding a matmul is produced in float32r.
"""
import numpy as np

import concourse.bacc as bacc
import concourse.bass as bass
import concourse.mybir as mybir
import concourse.tile as tile
from concourse.bass_utils import run_bass_kernel_spmd

F32 = mybir.dt.float32
F32R = mybir.dt.float32r
MUL = mybir.AluOpType.mult
SUB = mybir.AluOpType.subtract
ADD = mybir.AluOpType.add

N_CORES = 8
N, C, H, W = 64, 256, 32, 32
HW = H * W                      # 1024
N_LOC = N // N_CORES            # 8 samples per core
M_LOC = N_LOC * HW              # 8192
M_GLOB = N * HW                 # 65536
K_TILES = M_LOC // 128          # 64
EPS = 1e-5
T_ITERS = 10
RG = [list(range(N_CORES))]

_CACHED_NC = None


def build():
    nc = bacc.Bacc("TRN2", target_bir_lowering=False, debug=False,
                   num_devices=N_CORES)
    X = nc.dram_tensor("X", [N_LOC, C, HW], F32, kind="ExternalInput")
    ROT = nc.dram_tensor("rot", [C, C], F32, kind="ExternalInput")
    # aux[:, 0:256]   = identity tile 0 (col c == partition p)
    # aux[:, 256:512] = identity tile 1 (col c == 128 + p)
    # aux[:, 512:640] = all-ones block
    AUX = nc.dram_tensor("aux", [128, 640], F32R, kind="ExternalInput")
    OUT = nc.dram_tensor("out", [N_LOC, C, HW], F32, kind="ExternalOutput")

    with tile.TileContext(nc) as tc:
        _body(nc, tc, X, ROT, AUX, OUT)
    # Register the prelude 1-byte AllGather (bir_kernel_barrier) so the
    # cross-core first-collective rendezvous runs at kernel start,
    # overlapped with the local G phase, instead of serializing before the
    # AllReduce (collectives execute in issue order on the CC stream).
    nc._bir_kernel_barrier_sem_replica_groups.extend(set(g) for g in RG)
    nc.compile()
    return nc


def _body(nc, tc, X, ROT, AUX, OUT):
    ts = bass.ts

    with (
        tc.tile_pool(name="dram", bufs=1, space="DRAM") as dram,
        tc.tile_pool(name="const", bufs=1) as const,
        tc.tile_pool(name="xp", bufs=1) as xp,
        tc.tile_pool(name="xtp", bufs=6) as xtp,
        tc.tile_pool(name="nsp", bufs=1) as nsp,
        tc.tile_pool(name="pp", bufs=2) as pp,
        tc.tile_pool(name="outp", bufs=4) as outp,
    ):
        # ---------------- phase 0: input DMAs ---------------------------
        # x in float32r (SWDGE cast-DMA rounds), normal layout, 2 c-tiles
        xbuf = xp.tile([128, 2, M_LOC], F32R)
        for n in range(N_LOC):
            # dst[p, ct, hw] = X[n, ct*128 + p, hw]
            nc.gpsimd.dma_start(
                xbuf[:, :, n * HW:(n + 1) * HW],
                X.ap()[n].rearrange("(ct p) hw -> p ct hw", ct=2))
        aux = const.tile([128, 640], F32R)
        nc.sync.dma_start(aux[:], AUX.ap())
        rot_sb = const.tile([128, 2, C], F32R)  # R rows: [p, ctd, c]
        nc.gpsimd.dma_start(rot_sb[:],
                            ROT.ap().rearrange("(ct p) c -> p ct c", ct=2))

        eye0 = aux[:, 0:128]                    # 128x128 identity (f32r)
        ones_col = aux[:, 512:513]
        ones_row = aux[0:1, 512:640]

        rotT = const.tile([128, 2, C], F32R)    # R^T: [p(=c), ctc, d]

        # ------------- phases 1-2: G/s accumulation + AllReduce ---------
        gs_sb = nsp.tile([128, 2, 257], F32)
        with (
            tc.tile_pool(name="ps_t", bufs=4, space="PSUM") as ps_t,
            tc.tile_pool(name="ps_g", bufs=1, space="PSUM") as ps_g,
        ):
            # R^T via PE transposes (off critical path)
            for ctd in range(2):
                pt = ps_t.tile([128, 256], F32R, name="ptk")
                for ctc in range(2):
                    nc.tensor.transpose(pt[:, ts(ctc, 128)],
                                        rot_sb[:, ctd, ts(ctc, 128)], eye0)
                nc.scalar.copy(rotT[:, :, ts(ctd, 128)],
                               pt[:].rearrange("p (c t) -> p c t", c=2))

            # psum cols 256/257 accumulate the column sums via ones columns
            # (258 keeps the fp32r moving dim even)
            gps = [ps_g.tile([128, 258], F32, name=f"gps{mt}")
                   for mt in range(2)]
            for k in range(K_TILES):
                ptk = ps_t.tile([128, 256], F32R, name="ptk")
                for ct in range(2):
                    nc.tensor.transpose(ptk[:, ts(ct, 128)],
                                        xbuf[:, ct, ts(k, 128)], eye0)
                xt = xtp.tile([128, 258], F32R, name="xt")
                if k % 2 == 0:
                    nc.vector.tensor_copy(xt[:, 0:256], ptk[:])
                else:
                    nc.scalar.copy(xt[:, 0:256], ptk[:])
                nc.gpsimd.memset(xt[:, 256:258].bitcast(F32), 1.0)
                for mt in range(2):
                    nc.tensor.matmul(gps[mt][:], xt[:, ts(mt, 128)], xt[:],
                                     start=(k == 0), stop=(k == K_TILES - 1))

            # evict with a 1/m scale: the AllReduce then directly yields
            # G/m in cols 0:256 and mu in col 256
            inv_m = 1.0 / M_GLOB
            nc.scalar.activation(gs_sb[:, 0, :], gps[0][:, 0:257],
                                 mybir.ActivationFunctionType.Copy,
                                 scale=inv_m)
            nc.scalar.activation(gs_sb[:, 1, :], gps[1][:, 0:257],
                                 mybir.ActivationFunctionType.Copy,
                                 scale=inv_m)

        ar_in = dram.tile([128, 2, 257], F32)
        ar_out = dram.tile([128, 2, 257], F32, addr_space="Shared")
        nc.sync.dma_start(ar_in[:], gs_sb[:])
        nc.gpsimd.collective_compute(
            "AllReduce", mybir.AluOpType.add,
            replica_groups=RG, ins=[ar_in.opt()], outs=[ar_out.opt()],
        )
        ssb = nsp.tile([128, 2, 257], F32)
        nc.sync.dma_start(ssb[:], ar_out[:])

        # ------------- phase 3: Sigma, trace, scalars -------------------
        # ssb already holds G/m (cols 0:256) and mu (col 256)
        mu = nsp.tile([128, 4], F32R)      # cols 0,1 = mu; cols 2,3 = zero
        mu_neg = nsp.tile([128, 2], F32)
        mu_row = nsp.tile([1, 256], F32R)
        mu_row_b = nsp.tile([128, 256], F32R)
        sig = nsp.tile([128, 2, C], F32)
        sig_h = nsp.tile([128, 2, C], F32R)
        diag = nsp.tile([128, 2], F32)
        tr2 = nsp.tile([128, 2], F32)
        tr_col = nsp.tile([128, 1], F32)
        rec_col = nsp.tile([128, 1], F32)
        half_col = nsp.tile([128, 1], F32)
        sqrt_col = nsp.tile([128, 1], F32)
        epsh_col = nsp.tile([128, 1], F32)
        junk = nsp.tile([128, C], F32)
        rotTs = const.tile([128, 2, C], F32R)
        p_cur = pp.tile([128, 2, C], F32R, name="p")

        def eyef(mt):
            return aux[:, mt * 256:(mt + 1) * 256].bitcast(F32)

        with tc.tile_pool(name="ps3", bufs=1, space="PSUM") as ps3:
            nc.vector.tensor_copy(mu[:, 0:2], ssb[:, :, 256])
            nc.gpsimd.memset(mu[:, 2:4].bitcast(F32), 0.0)
            nc.vector.tensor_scalar_mul(mu_neg[:], ssb[:, :, 256], -1.0)
            # mu as a row on partition 0 (strided gather from the reduced
            # DRAM buffer: element [c', ct, 256] -> row position ct*128+c'),
            # then broadcast to all partitions
            nc.gpsimd.dma_start(
                mu_row[:].rearrange("a (ct c) -> a ct c", ct=2),
                ar_out[:, :, 256:257].rearrange("c ct one -> one ct c"))
            nc.gpsimd.partition_broadcast(mu_row_b[:], mu_row[:])

            # Sigma0 = G/m - mu mu^T ; diag via masked row-sums
            for mt in range(2):
                nc.vector.scalar_tensor_tensor(
                    sig[:, mt, :], mu_row_b[:].bitcast(F32),
                    mu_neg[:, mt:mt + 1], ssb[:, mt, 0:256],
                    op0=MUL, op1=ADD)
                nc.vector.scalar_tensor_tensor(
                    junk[:], sig[:, mt, :], 1.0, eyef(mt),
                    op0=MUL, op1=MUL, accum_out=diag[:, mt:mt + 1])
            # trace (+eps*256) replicated on all partitions, then scalars
            import concourse.bass_isa as bass_isa
            nc.gpsimd.partition_all_reduce(tr2[:], diag[:], channels=128,
                                           reduce_op=bass_isa.ReduceOp.add)
            nc.vector.scalar_tensor_tensor(
                tr_col[:], tr2[:, 0:1], 256.0 * EPS, tr2[:, 1:2],
                op0=ADD, op1=ADD)
            nc.vector.reciprocal(rec_col[:], tr_col[:])
            nc.vector.tensor_scalar_mul(half_col[:], rec_col[:], 0.5)
            nc.scalar.sqrt(sqrt_col[:], rec_col[:])
            nc.vector.tensor_scalar_mul(epsh_col[:], half_col[:], EPS)

            # Sig_h = 0.5/tr * (Sigma0 + eps I);  P1 = 1.5 I - Sig_h
            for mt in range(2):
                eye_sc = nsp.tile([128, C], F32, name=f"eye_sc{mt}")
                nc.scalar.activation(eye_sc[:], eyef(mt),
                                     mybir.ActivationFunctionType.Copy,
                                     scale=epsh_col[:])
                nc.vector.scalar_tensor_tensor(
                    sig_h[:, mt, :], sig[:, mt, :], half_col[:], eye_sc[:],
                    op0=MUL, op1=ADD)
                nc.vector.scalar_tensor_tensor(
                    p_cur[:, mt, :], eyef(mt), 1.5,
                    sig_h[:, mt, :].bitcast(F32),
                    op0=MUL, op1=SUB)

            # rotTs = R^T * sqrt(1/tr)  (fold the wm scale into rotation)
            for ct in range(2):
                nc.vector.tensor_scalar_mul(rotTs[:, ct, :],
                                            rotT[:, ct, :].bitcast(F32),
                                            sqrt_col[:])

        # ------------- phase 4: Newton-Schulz iterations 2..10 ----------
        # P_{k+1} = 1.5 P - (P P)(P Sig_h)  [iterates symmetric, commute]
        # T1 and T2 share one PSUM bank: T1's start=True clears the bank,
        # T2's first matmul (start=False) overwrites where has_written is
        # clear — one [128,512] eviction per mt instead of two.
        t12sb = nsp.tile([128, 2, 512], F32R)
        at_sb = nsp.tile([128, 2, C], F32R)
        negb = nsp.tile([128, 2], F32)
        with tc.tile_pool(name="ps4", bufs=1, space="PSUM") as ps4:
            for it in range(1, T_ITERS):
                t12ps = [ps4.tile([128, 512], F32, name=f"t12ps{mt}")
                         for mt in range(2)]
                for mt in range(2):
                    for ct in range(2):
                        nc.tensor.matmul(t12ps[mt][:, 0:256],
                                         p_cur[:, ct, ts(mt, 128)],
                                         p_cur[:, ct, :],
                                         start=(ct == 0), stop=False,
                                         skip_group_check=True)
                    for ct in range(2):
                        nc.tensor.matmul(t12ps[mt][:, 256:512],
                                         p_cur[:, ct, ts(mt, 128)],
                                         sig_h[:, ct, :],
                                         start=False, stop=(ct == 1),
                                         skip_group_check=True)
                for mt in range(2):
                    if mt == 0:
                        nc.vector.tensor_copy(t12sb[:, mt, :], t12ps[mt][:])
                    else:
                        nc.scalar.copy(t12sb[:, mt, :], t12ps[mt][:])
                p_new = pp.tile([128, 2, C], F32R, name="p")
                for mt in range(2):
                    t3ps = ps4.tile([128, C], F32, name=f"t3ps{mt}")
                    for ct in range(2):
                        nc.tensor.matmul(t3ps[:],
                                         t12sb[:, ct, ts(mt, 128)],
                                         t12sb[:, ct, 256:512],
                                         start=(ct == 0), stop=(ct == 1))
                    nc.vector.scalar_tensor_tensor(
                        p_new[:, mt, :], p_cur[:, mt, :].bitcast(F32), 1.5,
                        t3ps[:], op0=MUL, op1=SUB)
                p_cur = p_new

            # --------- phase 5: A^T = P10 @ rotTs, -b = -A mu -----------
            for mt in range(2):
                aps = ps4.tile([128, C], F32, name=f"t3ps{mt}")
                for ct in range(2):
                    nc.tensor.matmul(aps[:], p_cur[:, ct, ts(mt, 128)],
                                     rotTs[:, ct, :],
                                     start=(ct == 0), stop=(ct == 1))
                nc.vector.tensor_copy(at_sb[:, mt, :], aps[:])
            for mt in range(2):
                # N=2 keeps the fp32r moving dim even; col 1 is junk
                bps = ps4.tile([128, 2], F32, name=f"bps{mt}")
                for ct in range(2):
                    nc.tensor.matmul(bps[:], at_sb[:, ct, ts(mt, 128)],
                                     mu[:, ct:ct + 2],
                                     start=(ct == 0), stop=(ct == 1))
                nc.vector.tensor_scalar_mul(negb[:, mt:mt + 1], bps[:, 0:1],
                                            -1.0)

        # ------------- phase 6: apply + output --------------------------
        with tc.tile_pool(name="ps_o", bufs=4, space="PSUM") as ps_o:
            for j in range(2 * N_LOC):
                n, half = j // 2, j % 2
                m0 = n * HW + half * 512
                for mt in range(2):
                    ops = ps_o.tile([128, 512], F32, name="ops")
                    for ct in range(2):
                        nc.tensor.matmul(ops[:], at_sb[:, ct, ts(mt, 128)],
                                         xbuf[:, ct, m0:m0 + 512],
                                         start=(ct == 0), stop=(ct == 1))
                    osb = outp.tile([128, 512], F32, name="osb")
                    if (j + mt) % 2 == 0:
                        nc.vector.tensor_scalar_add(osb[:], ops[:],
                                                    negb[:, mt:mt + 1])
                    else:
                        nc.scalar.activation(
                            osb[:], ops[:],
                            mybir.ActivationFunctionType.Identity,
                            bias=negb[:, mt:mt + 1])
                    nc.sync.dma_start(
                        OUT.ap()[n, mt * 128:(mt + 1) * 128,
                                 half * 512:(half + 1) * 512],
                        osb[:])


def _aux_np():
    aux = np.zeros((128, 640), dtype=np.float32)
    aux[np.arange(128), np.arange(128)] = 1.0
    aux[np.arange(128), 256 + 128 + np.arange(128)] = 1.0
    aux[:, 512:640] = 1.0
    return aux


def kernel(X, running_rot):
    global _CACHED_NC
    X = np.ascontiguousarray(X, dtype=np.float32)
    rot = np.ascontiguousarray(
        np.asarray(running_rot, dtype=np.float32).reshape(C, C))
    aux = _aux_np()
    if _CACHED_NC is None:
        _CACHED_NC = build()
    nc = _CACHED_NC
    in_maps = []
    for c in range(N_CORES):
        shard = np.ascontiguousarray(
            X[c * N_LOC:(c + 1) * N_LOC].reshape(N_LOC, C, HW))
        in_maps.append({"X": shard, "rot": rot, "aux": aux})
    res = run_bass_kernel_spmd(nc, in_maps, list(range(N_CORES)))
    out = np.empty((N, C, H, W), dtype=np.float32)
    for c in range(N_CORES):
        out[c * N_LOC:(c + 1) * N_LOC] = \
            res.results[c]["out"].reshape(N_LOC, C, H, W)
    return out
